# revision 4
# baseline (speedup 1.0000x reference)
"""Trainium2 Bass kernel: contrastive loss with negative mining (v2).

Math:
    centers  = mean over contiguous chunks of 8 rows               [n/8, d]
    x_pos    = x + 0.5*(center - x)        => |x - x_pos| = 0.5*|x - center|
    sim      = x @ x.T                                             [n, n]
    neg_idx  = argmax_j sim[i, j] excluding j in i's group-of-4
    d_ap     = mean_d |x - x_pos|,  d_an = mean_d |x - x_neg|
    loss     = sum( (1/8) * d_ap / (d_an + 1e-7) )

Distribution: data-parallel over rows, 8 NeuronCores, 1024 rows each,
no collectives; per-row losses summed on host.

v2 mining strategy (packed argmax):
  - Each sim strip [128,512] is evacuated from PSUM by one ScalarE Copy
    activation computing t = 2^36 + 8192*round(2*v + 1024) -- the 2^36
    magic constant forces f32 rounding at ULP 8192, i.e. an 11-bit
    quantization of the sim value in the high bits.
  - One DVE scalar_tensor_tensor computes q = (t - 2^36) + in1 where
    in1 = 16*local_col + global_strip_id is a per-strip host table, so
    q = 8192*r + 16*col + s  (exact in f32, <= 2^24-1).
  - One DVE max-reduce per strip extracts the packed (value,index)
    candidate; a per-i-tile max over 16 candidates + exact u32 bit ops
    decode the global argmax column.  Group-of-4 exclusion windows are
    pre-baked as -2^26 into the in1 tables of the diagonal block.
  - Per-core xm drops the core's own 1024 columns (14 strips); the
    diagonal block is computed from the resident xs slice, so the PE
    starts ~2us after launch while xm streams in.
"""

import math

import ml_dtypes
import numpy as np

import concourse.bass as bass
import concourse.mybir as mybir
import concourse.tile as tile
from concourse import bacc
from concourse.bass import IndirectOffsetOnAxis
from concourse.bass_utils import run_bass_kernel_spmd

BF16 = mybir.dt.bfloat16
F32 = mybir.dt.float32
U32 = mybir.dt.uint32
ALU = mybir.AluOpType
ACTF = mybir.ActivationFunctionType
AXX = mybir.AxisListType.X

P = 128         # partitions / row-tile height
JS = 512        # similarity column-strip width
CHUNK = 8       # rows averaged per center
GROUP = 4       # negative-mining exclusion window
WEIGHT = 1.0 / 8
EPS = 1e-7

# packed-argmax constants: q = 8192*round(S*v + RB) + 16*col + strip
S = 2.0          # sim quantization scale (step 0.5, |v| < 511 safe)
RB = 1024.0      # round bias making r nonnegative
MAGIC = float(2 ** 36)            # f32 ULP at 2^36 is 8192
ABIAS = MAGIC + 8192.0 * RB       # activation bias
ASCALE = 8192.0 * S               # activation scale
MASKV = -float(2 ** 26)           # exclusion-window additive mask


class Cfg:
    def __init__(self, n=8192, d=2048, cores=8, fp8=True):
        self.n, self.d, self.cores, self.fp8 = n, d, cores, fp8
        self.r = n // cores            # rows per core
        self.it = self.r // P          # i-tiles per core
        self.nj = n // JS              # global column strips
        self.njr = self.nj - 2         # regular (non-diagonal) strips
        self.kb = d // P               # contraction blocks
        self.cw = min(d, JS)           # d-chunk width for the d_ap matmul
        self.ch = d // self.cw         # number of d-chunks
        self.gi = min(4, self.it)      # i-tiles per pass
        assert n % (cores * P) == 0 and d % P == 0 and n % JS == 0
        assert d % self.cw == 0 and self.it % self.gi == 0
        assert self.r == 2 * JS        # diag block is exactly 2 strips


def _body(tc: tile.TileContext, cfg: Cfg, io: dict):
    nc = tc.nc
    ctxpools = {}

    def pool(name, bufs, space="SBUF"):
        if name not in ctxpools:
            ctxpools[name] = tc.alloc_tile_pool(name=name, bufs=bufs, space=space)
        return ctxpools[name]

    sim_dt = mybir.dt.float8e4 if cfg.fp8 else BF16

    # resident stationary xT slice: [128, KB*R], k-block major.
    # Chunked DMAs so the diag matmuls start before the full load lands.
    xs_sb = pool("xs", 1).tile([P, cfg.kb * cfg.r], sim_dt, name="xs_sb")
    for k in range(0, cfg.kb, 2):
        ke = min(k + 2, cfg.kb)
        nc.sync.dma_start(
            out=xs_sb[:, k * cfg.r:ke * cfg.r].rearrange(
                "p (a r) -> p a r", a=ke - k),
            in_=io["xs"][k * P:ke * P, :].rearrange("(a p) r -> p a r", p=P),
        )

    consts = pool("consts", 1)
    dgw_sb = consts.tile_from(io["dgw"])        # [128, IT*2*512] f32 diag in1
    in1_sb = consts.tile_from(io["in1reg"])     # [128, NJR*512] f32
    m2b_sb = consts.tile_from(io["m2b"])        # [128,128] bf16

    # resident bf16 x rows (d_an minuend / d_ap input); needed only by the
    # pass tails, so the DMA is emitted here but lands behind xs/tables.
    xrb_sb = pool("xrb", 1).tile([P, cfg.it * cfg.d], BF16, name="xrb_sb")
    nc.sync.dma_start(
        out=xrb_sb[:].rearrange("p (a d) -> p a d", a=cfg.it),
        in_=io["xrb"][:, :].rearrange("(a p) d -> p a d", p=P),
    )

    psum = pool("ps", 8, space="PSUM")
    small = pool("small", 1)
    san = small.tile([P, cfg.it], F32, name="san")             # sum|x-xneg|
    sap = small.tile([P, cfg.it * cfg.ch], F32, name="sap")    # sum|y| per chunk
    idxall = small.tile([P, cfg.it], U32, name="idxall")       # neg indices
    npass = cfg.it // cfg.gi
    G = cfg.gi
    NS = cfg.nj  # candidate slots per i-tile (14 regular + 2 diag)
    cands = [small.tile([P, G * NS], F32, name=f"cd{g}", tag=f"cd{g}")
             for g in range(npass)]

    xmp = pool("xm", 3)
    qp = pool("q", 4)
    qqp = pool("qq", 4)
    comb = pool("comb", 1)
    xneg_p = pool("xneg", 2)
    diff_p = pool("diff", 2)
    dabs_p = pool("dabs", 2)

    xs3 = xs_sb[:].rearrange("p (a r) -> p a r", a=cfg.kb)

    def pack_and_reduce(ps_s, in1_ap, cd_slot):
        """PSUM strip -> packed f32 -> candidate slot."""
        q = qp.tile([P, JS], F32, name="q")
        nc.scalar.activation(out=q[:], in_=ps_s[:], func=ACTF.Copy,
                             scale=ASCALE, bias=ABIAS)
        qq = qqp.tile([P, JS], F32, name="qq")
        nc.vector.scalar_tensor_tensor(
            out=qq[:], in0=q[:], scalar=MAGIC, in1=in1_ap,
            op0=ALU.subtract, op1=ALU.add)
        nc.vector.tensor_reduce(out=cd_slot, in_=qq[:], axis=AXX, op=ALU.max)

    for a in range(0, cfg.it, G):
        g = a // G
        # ---- diagonal block from resident xs (starts immediately) ----
        for it in range(a, a + G):
            for dd in range(2):
                ps_s = psum.tile([P, JS], F32, name="ps_s", tag="ps")
                if cfg.fp8:
                    for k in range(0, cfg.kb, 2):
                        nc.tensor.matmul(
                            out=ps_s[:],
                            lhsT=xs3[:, k:k + 2, it * P:(it + 1) * P],
                            rhs=xs3[:, k:k + 2, dd * JS:(dd + 1) * JS],
                            start=(k == 0), stop=(k == cfg.kb - 2),
                            perf_mode=mybir.MatmulPerfMode.DoubleRow,
                        )
                else:
                    for k in range(cfg.kb):
                        nc.tensor.matmul(
                            out=ps_s[:],
                            lhsT=xs_sb[:, k * cfg.r + it * P:
                                       k * cfg.r + (it + 1) * P],
                            rhs=xs_sb[:, k * cfg.r + dd * JS:
                                      k * cfg.r + (dd + 1) * JS],
                            start=(k == 0), stop=(k == cfg.kb - 1),
                        )
                w0 = (it * 2 + dd) * JS
                pack_and_reduce(
                    ps_s, dgw_sb[:, w0:w0 + JS],
                    cands[g][:, (it - a) * NS + cfg.njr + dd:
                             (it - a) * NS + cfg.njr + dd + 1])

        # ---- regular strips streamed from packed xm ----
        for j in range(cfg.njr):
            xm_sb = xmp.tile([P, cfg.kb * JS], sim_dt, name="xm_sb")
            nc.sync.dma_start(
                out=xm_sb[:].rearrange("p (a b) -> p a b", a=cfg.kb),
                in_=io["xm"][:, j * JS:(j + 1) * JS].rearrange(
                    "(a p) b -> p a b", p=P),
            )
            xm3 = xm_sb[:].rearrange("p (a b) -> p a b", a=cfg.kb)
            for it in range(a, a + G):
                ps_s = psum.tile([P, JS], F32, name="ps_s", tag="ps")
                if cfg.fp8:
                    for k in range(0, cfg.kb, 2):
                        nc.tensor.matmul(
                            out=ps_s[:],
                            lhsT=xs3[:, k:k + 2, it * P:(it + 1) * P],
                            rhs=xm3[:, k:k + 2, :],
                            start=(k == 0), stop=(k == cfg.kb - 2),
                            perf_mode=mybir.MatmulPerfMode.DoubleRow,
                        )
                else:
                    for k in range(cfg.kb):
                        nc.tensor.matmul(
                            out=ps_s[:],
                            lhsT=xs_sb[:, k * cfg.r + it * P:
                                       k * cfg.r + (it + 1) * P],
                            rhs=xm_sb[:, k * JS:(k + 1) * JS],
                            start=(k == 0), stop=(k == cfg.kb - 1),
                        )
                pack_and_reduce(
                    ps_s, in1_sb[:, j * JS:(j + 1) * JS],
                    cands[g][:, (it - a) * NS + j:(it - a) * NS + j + 1])

        # ---- batched combine: decode argmax for this pass's i-tiles ----
        cd3 = cands[g][:].rearrange("p (g q) -> p g q", g=G)
        cq = comb.tile([P, G], F32, name="cq")
        nc.vector.tensor_reduce(out=cq[:], in_=cd3, axis=AXX, op=ALU.max)
        cqu = comb.tile([P, G], U32, name="cqu")
        nc.vector.tensor_copy(out=cqu[:], in_=cq[:])
        rem = comb.tile([P, G], U32, name="rem")
        nc.vector.tensor_scalar(
            out=rem[:], in0=cqu[:], scalar1=8191, scalar2=None,
            op0=ALU.bitwise_and)
        sid = comb.tile([P, G], U32, name="sid")
        nc.vector.tensor_scalar(
            out=sid[:], in0=rem[:], scalar1=15, scalar2=None,
            op0=ALU.bitwise_and)
        lid = comb.tile([P, G], U32, name="lid")
        nc.vector.tensor_scalar(
            out=lid[:], in0=rem[:], scalar1=4, scalar2=None,
            op0=ALU.logical_shift_right)
        s9 = comb.tile([P, G], U32, name="s9")
        nc.vector.tensor_scalar(
            out=s9[:], in0=sid[:], scalar1=9, scalar2=None,
            op0=ALU.logical_shift_left)
        nc.vector.tensor_tensor(
            out=idxall[:, a:a + G], in0=s9[:], in1=lid[:], op=ALU.bitwise_or)

        # ---- gather x_neg (bf16) + d_an for this pass's i-tiles ----
        for it in range(a, a + G):
            xneg = xneg_p.tile([P, cfg.d], BF16, name="xneg")
            nc.gpsimd.indirect_dma_start(
                out=xneg[:], out_offset=None,
                in_=io["xfb"][:, :],
                in_offset=IndirectOffsetOnAxis(ap=idxall[:, it:it + 1], axis=0),
                # an OOB index must not fault the device; skip it instead
                bounds_check=cfg.n - 1, oob_is_err=False,
            )
            diff = diff_p.tile([P, cfg.d], BF16, name="diff")
            nc.vector.tensor_tensor(
                out=diff[:], in0=xrb_sb[:, it * cfg.d:(it + 1) * cfg.d],
                in1=xneg[:], op=ALU.subtract,
            )
            dabs = dabs_p.tile([P, cfg.d], BF16, name="dabs")
            nc.scalar.activation(
                out=dabs[:], in_=diff[:], func=ACTF.Abs,
                accum_out=san[:, it:it + 1],
            )

    # ---- d_ap (emitted last; PE work overlaps the final pass's tail):
    #      y = M2 @ x_tile, sum_d |y|  (bf16 matmuls) ----
    yabs = pool("yabs", 2)
    for it in range(cfg.it):
        for c in range(cfg.ch):
            ps_y = psum.tile([P, cfg.cw], F32, name="ps_y", tag="ps")
            nc.tensor.matmul(
                out=ps_y[:], lhsT=m2b_sb[:],
                rhs=xrb_sb[:, it * cfg.d + c * cfg.cw:
                           it * cfg.d + (c + 1) * cfg.cw],
                start=True, stop=True,
            )
            y_sc = yabs.tile([P, cfg.cw], F32, name="y_sc")
            nc.scalar.activation(
                out=y_sc[:], in_=ps_y[:], func=ACTF.Abs,
                accum_out=sap[:, it * cfg.ch + c: it * cfg.ch + c + 1],
            )

    # ---- Final: per-row loss ----
    fin = pool("fin", 1)
    sap8 = fin.tile([P, cfg.it], F32, name="sap8")
    sap3 = sap[:].rearrange("p (a b) -> p a b", a=cfg.it)
    nc.vector.tensor_reduce(out=sap8[:], in_=sap3, axis=AXX, op=ALU.add)
    t1 = fin.tile([P, cfg.it], F32, name="t1")
    nc.vector.tensor_scalar(
        out=t1[:], in0=san[:], scalar1=1.0 / cfg.d, scalar2=EPS,
        op0=ALU.mult, op1=ALU.add,
    )
    rec = fin.tile([P, cfg.it], F32, name="rec")
    nc.vector.reciprocal(out=rec[:], in_=t1[:])
    t2 = fin.tile([P, cfg.it], F32, name="t2")
    nc.vector.tensor_tensor(out=t2[:], in0=sap8[:], in1=rec[:], op=ALU.mult)
    lossv = fin.tile([P, cfg.it], F32, name="lossv")
    nc.vector.tensor_scalar(
        out=lossv[:], in0=t2[:], scalar1=0.5 * WEIGHT / cfg.d, scalar2=None,
        op0=ALU.mult,
    )
    nc.sync.dma_start(out=io["loss_part"][:, :], in_=lossv[:])
    nc.sync.dma_start(out=io["nidx"][:, :], in_=idxall[:])

    for p in reversed(list(ctxpools.values())):
        p.release()


def build(cfg: Cfg) -> bass.Bass:
    nc = bacc.Bacc("TRN2", target_bir_lowering=False, debug=False)
    sim_dt = mybir.dt.float8e4 if cfg.fp8 else BF16
    io = {
        "xm": nc.dram_tensor("xm", [cfg.d, cfg.njr * JS], sim_dt,
                             kind="ExternalInput").ap(),
        "xs": nc.dram_tensor("xs", [cfg.d, cfg.r], sim_dt,
                             kind="ExternalInput").ap(),
        "xrb": nc.dram_tensor("xrb", [cfg.r, cfg.d], BF16,
                              kind="ExternalInput").ap(),
        "xfb": nc.dram_tensor("xfb", [cfg.n, cfg.d], BF16,
                              kind="ExternalInput").ap(),
        "m2b": nc.dram_tensor("m2b", [P, P], BF16, kind="ExternalInput").ap(),
        "in1reg": nc.dram_tensor("in1reg", [P, cfg.njr * JS], F32,
                                 kind="ExternalInput").ap(),
        "dgw": nc.dram_tensor("dgw", [P, cfg.it * 2 * JS], F32,
                              kind="ExternalInput").ap(),
        "loss_part": nc.dram_tensor("loss_part", [P, cfg.it], F32,
                                    kind="ExternalOutput").ap(),
        "nidx": nc.dram_tensor("nidx", [P, cfg.it], U32,
                               kind="ExternalOutput").ap(),
    }
    with tile.TileContext(nc) as tc:
        _body(tc, cfg, io)
    nc.compile()
    return nc


def make_in_maps(cfg: Cfg, x: np.ndarray) -> list[dict]:
    x = np.ascontiguousarray(x, dtype=np.float32)
    sim_np = ml_dtypes.float8_e4m3 if cfg.fp8 else ml_dtypes.bfloat16
    xt_q = np.ascontiguousarray(x.T.astype(sim_np))
    x_bf = x.astype(ml_dtypes.bfloat16)

    m2 = np.eye(P, dtype=np.float32)
    for c in range(P // CHUNK):
        m2[c * CHUNK:(c + 1) * CHUNK, c * CHUNK:(c + 1) * CHUNK] -= 1.0 / CHUNK
    m2b = m2.astype(ml_dtypes.bfloat16)

    local16 = np.arange(JS, dtype=np.float32) * 16.0
    pvec = np.arange(P)

    in_maps = []
    for c in range(cfg.cores):
        cols = np.ones(cfg.n, dtype=bool)
        cols[c * cfg.r:(c + 1) * cfg.r] = False
        xm_nd = np.ascontiguousarray(xt_q[:, cols])          # [d, njr*JS]

        gstrips = [s for s in range(cfg.nj)
                   if s not in (2 * c, 2 * c + 1)]
        in1reg = np.empty((cfg.njr, JS), dtype=np.float32)
        for jj, s in enumerate(gstrips):
            in1reg[jj] = local16 + s
        in1reg_t = np.broadcast_to(
            in1reg.reshape(-1), (P, cfg.njr * JS)).copy()

        dgw = np.empty((P, cfg.it, 2, JS), dtype=np.float32)
        for it in range(cfg.it):
            for dd in range(2):
                tile_ = np.broadcast_to(
                    local16 + (2 * c + dd), (P, JS)).copy()
                if dd == it // 4:
                    w0 = (it % 4) * P + (pvec - pvec % GROUP)
                    for off in range(GROUP):
                        tile_[pvec, w0 + off] += MASKV
                dgw[:, it, dd, :] = tile_

        in_maps.append({
            "xm": xm_nd,
            "xs": np.ascontiguousarray(xt_q[:, c * cfg.r:(c + 1) * cfg.r]),
            "xrb": np.ascontiguousarray(x_bf[c * cfg.r:(c + 1) * cfg.r]),
            "xfb": x_bf,
            "m2b": m2b,
            "in1reg": in1reg_t,
            "dgw": dgw.reshape(P, -1),
        })
    return in_maps


def reduce_outputs(cfg: Cfg, results: list[dict]) -> np.ndarray:
    total = 0.0
    for res in results:
        total += float(res["loss_part"].astype(np.float64).sum())
    return np.float32(total)


def run(cfg: Cfg, x: np.ndarray, trace: bool = False):
    nc = build(cfg)
    in_maps = make_in_maps(cfg, x)
    out = run_bass_kernel_spmd(nc, in_maps, list(range(cfg.cores)), trace=trace)
    return out


def kernel(x: np.ndarray) -> np.ndarray:
    cfg = Cfg(n=8192, d=2048, cores=8)
    last_err = None
    for _ in range(3):
        try:
            out = run(cfg, x)
            return reduce_outputs(cfg, out.results)
        except Exception as e:  # transient device errors: rebuild + retry
            last_err = e
    raise last_err


# revision 6
# speedup vs baseline: 1.2123x; 1.2123x over previous
"""Trainium2 Bass kernel: contrastive loss with negative mining (v3).

Math:
    centers  = mean over contiguous chunks of 8 rows               [n/8, d]
    x_pos    = x + 0.5*(center - x)        => |x - x_pos| = 0.5*|x - center|
    sim      = x @ x.T                                             [n, n]
    neg_idx  = argmax_j sim[i, j] excluding j in i's group-of-4
    d_ap     = mean_d |x - x_pos|,  d_an = mean_d |x - x_neg|
    loss     = sum( (1/8) * d_ap / (d_an + 1e-7) )

Distribution: data-parallel over rows, 8 NeuronCores, 1024 rows each,
no collectives; per-row losses summed on host.

v3 mining strategy (minimal-energy, bf16):
  - Per-core xm drops the core's own 1024 columns (14 strips); the
    diagonal block is computed from the resident xs slice so the PE
    starts ~2us after launch while xm streams in.  The group-of-4
    exclusion window always falls in the diagonal block, where the
    python loop knows the i-tile, so masks are static bf16 tiles.
  - Each sim strip is evacuated PSUM->bf16 by ScalarE into a per-i-tile
    keep buffer; one DVE max-reduce yields the strip's top value.  The
    keep buffer is DMA'd to a DRAM scratch laid out so row p*128+it*16+
    slot holds (p, it, slot)'s strip.
  - Combine: MAX8+max_index over the 16 per-strip maxima give the
    winning slot; an indirect DMA gathers each row's winning strip from
    scratch; MAX8+max_index on that recovers the column; u32 arithmetic
    maps (slot, col) to the global index.  This needs no FIND_INDEX8
    over full strips (the big DVE cost) and keeps everything bf16 to
    stay off the power throttle.
"""

import math

import ml_dtypes
import numpy as np

import concourse.bass as bass
import concourse.mybir as mybir
import concourse.tile as tile
from concourse import bacc
from concourse.bass import IndirectOffsetOnAxis
from concourse.bass_utils import run_bass_kernel_spmd

BF16 = mybir.dt.bfloat16
F32 = mybir.dt.float32
U32 = mybir.dt.uint32
ALU = mybir.AluOpType
ACTF = mybir.ActivationFunctionType
AXX = mybir.AxisListType.X

P = 128         # partitions / row-tile height
JS = 512        # similarity column-strip width
CHUNK = 8       # rows averaged per center
GROUP = 4       # negative-mining exclusion window
WEIGHT = 1.0 / 8
EPS = 1e-7
MASKV = -float(2 ** 26)           # exclusion-window additive mask


class Cfg:
    def __init__(self, n=8192, d=2048, cores=8, fp8=True):
        self.n, self.d, self.cores, self.fp8 = n, d, cores, fp8
        self.r = n // cores            # rows per core
        self.it = self.r // P          # i-tiles per core
        self.nj = n // JS              # global column strips
        self.njr = self.nj - 2         # regular (non-diagonal) strips
        self.kb = d // P               # contraction blocks
        self.cw = min(d, JS)           # d-chunk width for the d_ap matmul
        self.ch = d // self.cw         # number of d-chunks
        self.gi = min(4, self.it)      # i-tiles per pass
        assert n % (cores * P) == 0 and d % P == 0 and n % JS == 0
        assert d % self.cw == 0 and self.it % self.gi == 0
        assert self.r == 2 * JS        # diag block is exactly 2 strips


def _body(tc: tile.TileContext, cfg: Cfg, io: dict):
    nc = tc.nc
    ctxpools = {}

    def pool(name, bufs, space="SBUF"):
        if name not in ctxpools:
            ctxpools[name] = tc.alloc_tile_pool(name=name, bufs=bufs, space=space)
        return ctxpools[name]

    sim_dt = mybir.dt.float8e4 if cfg.fp8 else BF16
    NS = cfg.nj          # slots per i-tile: 0..13 regular, 14..15 diag
    G = cfg.gi

    # resident stationary xT slice: [128, KB*R], k-block major.
    # Chunked DMAs so the diag matmuls start before the full load lands.
    xs_sb = pool("xs", 1).tile([P, cfg.kb * cfg.r], sim_dt, name="xs_sb")
    for k in range(0, cfg.kb, 2):
        ke = min(k + 2, cfg.kb)
        nc.sync.dma_start(
            out=xs_sb[:, k * cfg.r:ke * cfg.r].rearrange(
                "p (a r) -> p a r", a=ke - k),
            in_=io["xs"][k * P:ke * P, :].rearrange("(a p) r -> p a r", p=P),
        )

    consts = pool("consts", 1)
    maskdg_sb = consts.tile_from(io["maskdg"])   # [128, 4*512] bf16
    prow_sb = consts.tile_from(io["prow"])       # [128, 8] u32
    ctab2_sb = consts.tile_from(io["ctab2"])     # [128, 8] u32 (value 2c)
    m2b_sb = consts.tile_from(io["m2b"])         # [128,128] bf16

    # resident bf16 x rows (d_an minuend / d_ap input); needed only by the
    # pass tails, so its DMA is emitted after the small consts.
    xrb_sb = pool("xrb", 1).tile([P, cfg.it * cfg.d], BF16, name="xrb_sb")
    nc.sync.dma_start(
        out=xrb_sb[:].rearrange("p (a d) -> p a d", a=cfg.it),
        in_=io["xrb"][:, :].rearrange("(a p) d -> p a d", p=P),
    )

    psum = pool("ps", 8, space="PSUM")
    small = pool("small", 1)
    san = small.tile([P, cfg.it], F32, name="san")             # sum|x-xneg|
    sap = small.tile([P, cfg.it * cfg.ch], F32, name="sap")    # sum|y| per chunk
    idxall = small.tile([P, cfg.it], U32, name="idxall")       # neg indices
    npass = cfg.it // G
    cands = [small.tile([P, G * NS], BF16, name=f"cd{g}", tag=f"cd{g}")
             for g in range(npass)]

    xmp = pool("xm", 3)
    sskp = pool("ssk", 4)
    tmpd = pool("tmpd", 2)
    comb = pool("comb", 1)
    segp = pool("seg", 2)
    xneg_p = pool("xneg", 2)
    diff_p = pool("diff", 2)
    dabs_p = pool("dabs", 2)

    xs3 = xs_sb[:].rearrange("p (a r) -> p a r", a=cfg.kb)
    simr3 = io["simr"].rearrange("(p a) c -> p a c", p=P)   # [128,128,512]

    def sim_matmuls(ps_s, it, rhs_fp8, rhs_bf16_slice):
        if cfg.fp8:
            for k in range(0, cfg.kb, 2):
                nc.tensor.matmul(
                    out=ps_s[:],
                    lhsT=xs3[:, k:k + 2, it * P:(it + 1) * P],
                    rhs=rhs_fp8(k),
                    start=(k == 0), stop=(k == cfg.kb - 2),
                    perf_mode=mybir.MatmulPerfMode.DoubleRow,
                )
        else:
            for k in range(cfg.kb):
                nc.tensor.matmul(
                    out=ps_s[:],
                    lhsT=xs_sb[:, k * cfg.r + it * P:k * cfg.r + (it + 1) * P],
                    rhs=rhs_bf16_slice(k),
                    start=(k == 0), stop=(k == cfg.kb - 1),
                )

    for a in range(0, cfg.it, G):
        g = a // G
        ssks = {}
        # ---- diagonal block from resident xs (starts immediately) ----
        for it in range(a, a + G):
            ssk = sskp.tile([P, NS * JS], BF16, name="ssk")
            ssks[it] = ssk
            for dd in range(2):
                ps_s = psum.tile([P, JS], F32, name="ps_s", tag="ps")
                sim_matmuls(
                    ps_s, it,
                    lambda k, dd=dd: xs3[:, k:k + 2, dd * JS:(dd + 1) * JS],
                    lambda k, dd=dd: xs_sb[:, k * cfg.r + dd * JS:
                                           k * cfg.r + (dd + 1) * JS],
                )
                slot = cfg.njr + dd
                dst = ssk[:, slot * JS:(slot + 1) * JS]
                if dd == it // 4:
                    tmp = tmpd.tile([P, JS], BF16, name="tmp")
                    nc.scalar.copy(out=tmp[:], in_=ps_s[:])
                    nc.vector.tensor_tensor(
                        out=dst, in0=tmp[:],
                        in1=maskdg_sb[:, (it % 4) * JS:(it % 4 + 1) * JS],
                        op=ALU.add)
                else:
                    nc.scalar.copy(out=dst, in_=ps_s[:])
                nc.vector.tensor_reduce(
                    out=cands[g][:, (it - a) * NS + slot:
                                 (it - a) * NS + slot + 1],
                    in_=dst, axis=AXX, op=ALU.max)

        # ---- regular strips streamed from packed xm ----
        for j in range(cfg.njr):
            xm_sb = xmp.tile([P, cfg.kb * JS], sim_dt, name="xm_sb")
            nc.sync.dma_start(
                out=xm_sb[:].rearrange("p (a b) -> p a b", a=cfg.kb),
                in_=io["xm"][:, j * JS:(j + 1) * JS].rearrange(
                    "(a p) b -> p a b", p=P),
            )
            xm3 = xm_sb[:].rearrange("p (a b) -> p a b", a=cfg.kb)
            for it in range(a, a + G):
                ps_s = psum.tile([P, JS], F32, name="ps_s", tag="ps")
                sim_matmuls(
                    ps_s, it,
                    lambda k: xm3[:, k:k + 2, :],
                    lambda k: xm_sb[:, k * JS:(k + 1) * JS],
                )
                dst = ssks[it][:, j * JS:(j + 1) * JS]
                nc.scalar.copy(out=dst, in_=ps_s[:])
                nc.vector.tensor_reduce(
                    out=cands[g][:, (it - a) * NS + j:(it - a) * NS + j + 1],
                    in_=dst, axis=AXX, op=ALU.max)
            # stage keep-buffer writeback in two halves so the tail only
            # waits for the second half
            if j == 7:
                for it in range(a, a + G):
                    nc.sync.dma_start(
                        out=simr3[:, it * NS:it * NS + 8, :],
                        in_=ssks[it][:, 0:8 * JS].rearrange(
                            "p (s c) -> p s c", s=8),
                    )
        for it in range(a, a + G):
            nc.sync.dma_start(
                out=simr3[:, it * NS + 8:(it + 1) * NS, :],
                in_=ssks[it][:, 8 * JS:].rearrange(
                    "p (s c) -> p s c", s=NS - 8),
            )

        # ---- combine: find winning slot, recover column via gather ----
        slotv = comb.tile([P, G], U32, name="slotv", tag=f"slotv{g}")
        for it in range(a, a + G):
            t8 = comb.tile([P, 8], BF16, name="t8")
            nc.vector.max(
                out=t8[:], in_=cands[g][:, (it - a) * NS:(it - a + 1) * NS])
            i8 = comb.tile([P, 8], U32, name="i8")
            nc.vector.max_index(
                out=i8[:], in_max=t8[:],
                in_values=cands[g][:, (it - a) * NS:(it - a + 1) * NS])
            nc.vector.tensor_copy(
                out=slotv[:, it - a:it - a + 1], in_=i8[:, 0:1])
        rowidv = comb.tile([P, G], U32, name="rowidv", tag=f"row{g}")
        nc.vector.tensor_tensor(
            out=rowidv[:], in0=prow_sb[:, a:a + G], in1=slotv[:], op=ALU.add)

        # global strip id: g = slot + 2*(slot >= 2c)       (regular)
        #                  g = 2c + (slot - 14)            (diag slots)
        ge2 = comb.tile([P, G], U32, name="ge2")
        nc.vector.tensor_tensor(
            out=ge2[:], in0=slotv[:], in1=ctab2_sb[:, a:a + G], op=ALU.is_ge)
        ge2s = comb.tile([P, G], U32, name="ge2s")
        nc.vector.tensor_scalar(
            out=ge2s[:], in0=ge2[:], scalar1=1, scalar2=None,
            op0=ALU.logical_shift_left)
        gplus = comb.tile([P, G], U32, name="gplus")
        nc.vector.tensor_tensor(
            out=gplus[:], in0=slotv[:], in1=ge2s[:], op=ALU.add)
        dsum = comb.tile([P, G], U32, name="dsum")
        nc.vector.tensor_tensor(
            out=dsum[:], in0=slotv[:], in1=ctab2_sb[:, a:a + G], op=ALU.add)
        dgv = comb.tile([P, G], U32, name="dgv")
        nc.vector.tensor_scalar(
            out=dgv[:], in0=dsum[:], scalar1=cfg.njr, scalar2=None,
            op0=ALU.subtract)
        isd = comb.tile([P, G], U32, name="isd")
        nc.vector.tensor_scalar(
            out=isd[:], in0=slotv[:], scalar1=cfg.njr, scalar2=None,
            op0=ALU.is_ge)
        gsel = comb.tile([P, G], U32, name="gsel", tag=f"gsel{g}")
        nc.vector.select(out=gsel[:], mask=isd[:], on_true=dgv[:],
                         on_false=gplus[:])
        gsh = comb.tile([P, G], U32, name="gsh", tag=f"gsh{g}")
        nc.vector.tensor_scalar(
            out=gsh[:], in0=gsel[:], scalar1=9, scalar2=None,
            op0=ALU.logical_shift_left)

        colv = comb.tile([P, G], U32, name="colv", tag=f"colv{g}")
        for it in range(a, a + G):
            seg = segp.tile([P, JS], BF16, name="seg")
            nc.gpsimd.indirect_dma_start(
                out=seg[:], out_offset=None,
                in_=io["simr"][:, :],
                in_offset=IndirectOffsetOnAxis(
                    ap=rowidv[:, it - a:it - a + 1], axis=0),
                bounds_check=P * P - 1, oob_is_err=False,
            )
            s8 = comb.tile([P, 8], BF16, name="s8")
            nc.vector.max(out=s8[:], in_=seg[:])
            c8 = comb.tile([P, 8], U32, name="c8")
            nc.vector.max_index(out=c8[:], in_max=s8[:], in_values=seg[:])
            nc.vector.tensor_copy(
                out=colv[:, it - a:it - a + 1], in_=c8[:, 0:1])
        nc.vector.tensor_tensor(
            out=idxall[:, a:a + G], in0=gsh[:], in1=colv[:], op=ALU.bitwise_or)

        # ---- gather x_neg (bf16) + d_an for this pass's i-tiles ----
        for it in range(a, a + G):
            xneg = xneg_p.tile([P, cfg.d], BF16, name="xneg")
            nc.gpsimd.indirect_dma_start(
                out=xneg[:], out_offset=None,
                in_=io["xfb"][:, :],
                in_offset=IndirectOffsetOnAxis(ap=idxall[:, it:it + 1], axis=0),
                # an OOB index must not fault the device; skip it instead
                bounds_check=cfg.n - 1, oob_is_err=False,
            )
            diff = diff_p.tile([P, cfg.d], BF16, name="diff")
            nc.vector.tensor_tensor(
                out=diff[:], in0=xrb_sb[:, it * cfg.d:(it + 1) * cfg.d],
                in1=xneg[:], op=ALU.subtract,
            )
            dabs = dabs_p.tile([P, cfg.d], BF16, name="dabs")
            nc.scalar.activation(
                out=dabs[:], in_=diff[:], func=ACTF.Abs,
                accum_out=san[:, it:it + 1],
            )

    # ---- d_ap (emitted last; PE work overlaps the final pass's tail):
    #      y = M2 @ x_tile, sum_d |y|  (bf16 matmuls) ----
    yabs = pool("yabs", 2)
    for it in range(cfg.it):
        for c in range(cfg.ch):
            ps_y = psum.tile([P, cfg.cw], F32, name="ps_y", tag="ps")
            nc.tensor.matmul(
                out=ps_y[:], lhsT=m2b_sb[:],
                rhs=xrb_sb[:, it * cfg.d + c * cfg.cw:
                           it * cfg.d + (c + 1) * cfg.cw],
                start=True, stop=True,
            )
            y_sc = yabs.tile([P, cfg.cw], F32, name="y_sc")
            nc.scalar.activation(
                out=y_sc[:], in_=ps_y[:], func=ACTF.Abs,
                accum_out=sap[:, it * cfg.ch + c: it * cfg.ch + c + 1],
            )

    # ---- Final: per-row loss ----
    fin = pool("fin", 1)
    sap8 = fin.tile([P, cfg.it], F32, name="sap8")
    sap3 = sap[:].rearrange("p (a b) -> p a b", a=cfg.it)
    nc.vector.tensor_reduce(out=sap8[:], in_=sap3, axis=AXX, op=ALU.add)
    t1 = fin.tile([P, cfg.it], F32, name="t1")
    nc.vector.tensor_scalar(
        out=t1[:], in0=san[:], scalar1=1.0 / cfg.d, scalar2=EPS,
        op0=ALU.mult, op1=ALU.add,
    )
    rec = fin.tile([P, cfg.it], F32, name="rec")
    nc.vector.reciprocal(out=rec[:], in_=t1[:])
    t2 = fin.tile([P, cfg.it], F32, name="t2")
    nc.vector.tensor_tensor(out=t2[:], in0=sap8[:], in1=rec[:], op=ALU.mult)
    lossv = fin.tile([P, cfg.it], F32, name="lossv")
    nc.vector.tensor_scalar(
        out=lossv[:], in0=t2[:], scalar1=0.5 * WEIGHT / cfg.d, scalar2=None,
        op0=ALU.mult,
    )
    nc.sync.dma_start(out=io["loss_part"][:, :], in_=lossv[:])
    nc.sync.dma_start(out=io["nidx"][:, :], in_=idxall[:])

    for p in reversed(list(ctxpools.values())):
        p.release()


def build(cfg: Cfg) -> bass.Bass:
    nc = bacc.Bacc("TRN2", target_bir_lowering=False, debug=False)
    sim_dt = mybir.dt.float8e4 if cfg.fp8 else BF16
    io = {
        "xm": nc.dram_tensor("xm", [cfg.d, cfg.njr * JS], sim_dt,
                             kind="ExternalInput").ap(),
        "xs": nc.dram_tensor("xs", [cfg.d, cfg.r], sim_dt,
                             kind="ExternalInput").ap(),
        "xrb": nc.dram_tensor("xrb", [cfg.r, cfg.d], BF16,
                              kind="ExternalInput").ap(),
        "xfb": nc.dram_tensor("xfb", [cfg.n, cfg.d], BF16,
                              kind="ExternalInput").ap(),
        "m2b": nc.dram_tensor("m2b", [P, P], BF16, kind="ExternalInput").ap(),
        "maskdg": nc.dram_tensor("maskdg", [P, 4 * JS], BF16,
                                 kind="ExternalInput").ap(),
        "prow": nc.dram_tensor("prow", [P, 8], U32,
                               kind="ExternalInput").ap(),
        "ctab2": nc.dram_tensor("ctab2", [P, 8], U32,
                                kind="ExternalInput").ap(),
        "simr": nc.dram_tensor("simr", [P * P, JS], BF16,
                               kind="Internal").ap(),
        "loss_part": nc.dram_tensor("loss_part", [P, cfg.it], F32,
                                    kind="ExternalOutput").ap(),
        "nidx": nc.dram_tensor("nidx", [P, cfg.it], U32,
                               kind="ExternalOutput").ap(),
    }
    with tile.TileContext(nc) as tc:
        _body(tc, cfg, io)
    nc.compile()
    return nc


def make_in_maps(cfg: Cfg, x: np.ndarray) -> list[dict]:
    x = np.ascontiguousarray(x, dtype=np.float32)
    sim_np = ml_dtypes.float8_e4m3 if cfg.fp8 else ml_dtypes.bfloat16
    xt_q = np.ascontiguousarray(x.T.astype(sim_np))
    x_bf = x.astype(ml_dtypes.bfloat16)

    m2 = np.eye(P, dtype=np.float32)
    for c in range(P // CHUNK):
        m2[c * CHUNK:(c + 1) * CHUNK, c * CHUNK:(c + 1) * CHUNK] -= 1.0 / CHUNK
    m2b = m2.astype(ml_dtypes.bfloat16)

    pvec = np.arange(P)
    # mask tiles per it%4: -2^26 on the 4-column window, else 0
    maskdg = np.zeros((P, 4, JS), dtype=np.float32)
    for itv in range(4):
        w0 = itv * P + (pvec - pvec % GROUP)
        for off in range(GROUP):
            maskdg[pvec, itv, w0 + off] = MASKV
    maskdg_b = maskdg.reshape(P, -1).astype(ml_dtypes.bfloat16)

    # row-id base for the simr gather: p*128 + it*16
    prow = (pvec[:, None] * (P) + np.arange(8)[None, :] * 16).astype(np.uint32)

    in_maps = []
    for c in range(cfg.cores):
        cols = np.ones(cfg.n, dtype=bool)
        cols[c * cfg.r:(c + 1) * cfg.r] = False
        xm_nd = np.ascontiguousarray(xt_q[:, cols])          # [d, njr*JS]
        ctab2 = np.full((P, 8), 2 * c, dtype=np.uint32)

        in_maps.append({
            "xm": xm_nd,
            "xs": np.ascontiguousarray(xt_q[:, c * cfg.r:(c + 1) * cfg.r]),
            "xrb": np.ascontiguousarray(x_bf[c * cfg.r:(c + 1) * cfg.r]),
            "xfb": x_bf,
            "m2b": m2b,
            "maskdg": maskdg_b,
            "prow": prow,
            "ctab2": ctab2,
        })
    return in_maps


def reduce_outputs(cfg: Cfg, results: list[dict]) -> np.ndarray:
    total = 0.0
    for res in results:
        total += float(res["loss_part"].astype(np.float64).sum())
    return np.float32(total)


def run(cfg: Cfg, x: np.ndarray, trace: bool = False):
    nc = build(cfg)
    in_maps = make_in_maps(cfg, x)
    out = run_bass_kernel_spmd(nc, in_maps, list(range(cfg.cores)), trace=trace)
    return out


def kernel(x: np.ndarray) -> np.ndarray:
    cfg = Cfg(n=8192, d=2048, cores=8)
    last_err = None
    for _ in range(3):
        try:
            out = run(cfg, x)
            return reduce_outputs(cfg, out.results)
        except Exception as e:  # transient device errors: rebuild + retry
            last_err = e
    raise last_err
